# revision 3
# baseline (speedup 1.0000x reference)
"""Multi-head attention (B=2, S=2048, D=1024, H=16) on 8 Trainium2 NeuronCores.

Sharding: tensor-parallel over heads - 2 heads per core. Each core computes
its heads' QKV projection, attention, and a partial FC output (row-slice of
the FC contraction); the host sums the 8 partials and adds the FC bias
(plus the exact bv @ w_fc^T term -- attention is affine in V).

Fully software-pipelined single-pass schedule (~205 us vs 273 us baseline).
The ScalarE exp stream (~130 us of ACTIVATE) is the kernel clock; all other
work (projections, V-transposes, AV, FC, evictions, DMA) is interleaved
around it via a work queue drained between score-tile groups:
  - x is staged host-side as [tb, p, kt, t] so each token-block DMA moves
    8KB-contiguous partition lines (full DMA bandwidth; proj never waits).
  - K bias is dropped (softmax-invariant); V bias folded into the host
    b_fc add; Q bias and the 1/8 score scale fold into the Q eviction.
  - V reaches key-major layout via 3D-output DMA xbar transposes into a
    contiguous staging tile (non-contiguous xbar destinations are broken
    on HW) + a DVE scatter copy.
  - AV lhsT is [ones|V] / [V|ones] so the AV matmul also emits softmax
    denominators; reciprocal rows are broadcast across partitions by the
    (otherwise idle) GpSimd engine. Custom-DVE ops only work at base
    partition 0; one head's denominator row is moved there by DMA.
  - In the ACT-bound batch-1 phase, 2 of 16 exp tiles per step run on the
    DVE as a Schraudolph bit-trick (round(x*1477.32+15315.84) -> int16,
    bitcast to fp16; the -44.16 offset centers the piecewise-linear error).
  - NOTE (profiled at 205us): the true bottleneck is the TENSOR engine
    (173us active, 87.5%; ACT 133us, DVE 98us, Pool 17us; head ~14.5us of
    init+DMA-issue, ~12us of small PE gaps, ~6us tail). fp8-DoubleRow
    matmuls were tried and measured: proj x/w-fp8 and AV et/V-fp8 exceed
    the 2e-2 error gate (2.6e-2 / 1.9e-2 on this output, which cancels
    heavily through softmax averaging so quantization noise does not wash
    out); scores Q/K-fp8 DR passes accuracy (1.4e-2) but is SLOWER on HW
    (PE 190us) because 32-row DoubleRow ldweights dominate. OUT-fp16 and
    splitting startup DMA issue across sync+scalar queues also measured
    neutral-to-worse. This schedule is a strong local optimum.
  - CRITICAL: the tile framework derives dependencies from ISSUE order.
    Every queue item must be issued after its producers and before its
    consumers; see the deadline comments at the queue definitions. A
    violation reads uninitialized SBUF and is masked on warm reruns
    (stale memory holds the previous run's correct values).
"""
import numpy as np
from collections import deque
from contextlib import ExitStack

import concourse.bass as bass
import concourse.tile as tile
from concourse import bacc, mybir
from concourse.bass_utils import run_bass_kernel_spmd

B, S, D, H, HD = 2, 2048, 1024, 16, 64
T = B * S                # 4096 tokens
NC = 8                   # cores
HPC = H // NC            # heads per core
F = HPC * HD             # 128 value-features per core
KT = 128                 # key tile
QB = 512                 # query block
NKT = S // KT            # 16 key tiles per batch
NQB = S // QB            # 4 query blocks per batch
NTB = T // QB            # 8 token blocks
f32 = mybir.dt.float32
fp16 = mybir.dt.float16
AF = mybir.ActivationFunctionType
OP = mybir.AluOpType

AV_LAG = 4

_NC_CACHE = None
_DEBUG = False


def _build():
    nc = bacc.Bacc("TRN2", target_bir_lowering=False, debug=False, num_devices=NC)

    XB = nc.dram_tensor("xb", [NTB, 128, D // 128, QB], fp16, kind="ExternalInput").ap()
    WQ = nc.dram_tensor("wq", [128, D // 128, F], fp16, kind="ExternalInput").ap()
    WK = nc.dram_tensor("wk", [128, D // 128, F], fp16, kind="ExternalInput").ap()
    WV = nc.dram_tensor("wv", [128, D // 128, F], fp16, kind="ExternalInput").ap()
    BQ = nc.dram_tensor("bq", [F, 1], f32, kind="ExternalInput").ap()
    WFC = nc.dram_tensor("wfc", [F, D], fp16, kind="ExternalInput").ap()
    OUT = nc.dram_tensor("out", [T, D], f32, kind="ExternalOutput").ap()
    if _DEBUG:
        DBG_V = nc.dram_tensor("dbg_v", [B, 128, S], fp16,
                               kind="ExternalOutput").ap()
        DBG_DEN = nc.dram_tensor("dbg_den", [8, 3, QB], f32,
                                 kind="ExternalOutput").ap()
        DBG_FOUT = nc.dram_tensor("dbg_fout", [T // 128, 128, D], f32,
                                  kind="ExternalOutput").ap()
        DBG_RB = nc.dram_tensor("dbg_rb", [4, 128, D], f32,
                                kind="ExternalOutput").ap()
        DBG_VK = nc.dram_tensor("dbg_vk", [2, 128, NKT, 128], fp16,
                                kind="ExternalOutput").ap()
        DBG_KQV = nc.dram_tensor("dbg_kqv", [3, 128, S], fp16,
                                 kind="ExternalOutput").ap()
        DBG_ET = nc.dram_tensor("dbg_et", [NKT, 128, 2 * QB], fp16,
                                kind="ExternalOutput").ap()
        DBG_PAV = nc.dram_tensor("dbg_pav", [2, 128, QB], f32,
                                 kind="ExternalOutput").ap()

    with tile.TileContext(nc) as tc, ExitStack() as ctx:
        const = ctx.enter_context(tc.tile_pool(name="const", bufs=1))
        xt_pool = ctx.enter_context(tc.tile_pool(name="xt", bufs=1))
        big = ctx.enter_context(tc.tile_pool(name="big", bufs=1))
        vk_pool = ctx.enter_context(tc.tile_pool(name="vk", bufs=1))
        et_pool = ctx.enter_context(tc.tile_pool(name="et", bufs=8))
        nrm_pool = ctx.enter_context(tc.tile_pool(name="nrm", bufs=2))
        fout_pool = ctx.enter_context(tc.tile_pool(name="fout", bufs=6))
        ps_pool = ctx.enter_context(tc.tile_pool(name="ps", bufs=1, space="PSUM"))

        # --- constants / weights ---
        wq_sb = const.tile([128, D // 128, F], fp16)
        wk_sb = const.tile([128, D // 128, F], fp16)
        wv_sb = const.tile([128, D // 128, F], fp16)
        bq_sb = const.tile([F, 1], f32)
        wfc_sb = const.tile([F, D], fp16)

        xts = [xt_pool.tile([128, D // 128, QB], fp16, name=f"xts{i}")
               for i in range(NTB)]
        qT = [big.tile([128, S], fp16, name=f"qT{b}") for b in range(B)]
        kT = [big.tile([128, S], fp16, name=f"kT{b}") for b in range(B)]
        vT = [big.tile([128, S], fp16, name=f"vT{b}") for b in range(B)]
        valuesT = [big.tile([128, S], fp16, name=f"valuesT{b}") for b in range(B)]
        # vk[b][h]: [128 keys, NKT, 128] AV lhsT. h0 = [ones | V0] (den in psum
        # parts 0:64, values 64:128); h1 = [V1 | ones] (values 0:64, den 64:128).
        vk = [[vk_pool.tile([128, NKT, 128], fp16, name=f"vk{b}_{h}")
               for h in range(HPC)] for b in range(B)]

        # DMA order: first x block 0 (unblocks K proj), then weights, rest of x.
        nc.sync.dma_start(out=xts[0], in_=XB[0])
        nc.sync.dma_start(out=wk_sb, in_=WK)
        nc.sync.dma_start(out=wq_sb, in_=WQ)
        nc.sync.dma_start(out=bq_sb, in_=BQ)
        nc.sync.dma_start(out=xts[1], in_=XB[1])
        nc.sync.dma_start(out=wv_sb, in_=WV)
        nc.sync.dma_start(out=xts[2], in_=XB[2])
        nc.sync.dma_start(out=xts[3], in_=XB[3])
        nc.sync.dma_start(out=wfc_sb, in_=WFC)
        for i in range(4, NTB):
            nc.sync.dma_start(out=xts[i], in_=XB[i])

        # warm up the ACT exp table (~2.7us load) before the first real exp
        warm = const.tile([1, 1], f32)
        nc.scalar.activation(warm, bq_sb[0:1, 0:1], AF.Exp)

        # --- helpers (issue instructions; deps handled by tile framework) ---
        proj_ps = {}

        def proj_half(kind, b, tb, half):
            """Half a QKV projection group (4 of 8 contraction matmuls).
            Split so drained PE work injects <=1us bubbles into the
            score->exp stream."""
            w_sb, dst = {
                "q": (wq_sb, qT[b]), "k": (wk_sb, kT[b]), "v": (wv_sb, vT[b]),
            }[kind]
            tq = (tb % NQB) * QB
            if half == 0:
                ps = ps_pool.tile([128, QB], f32, tag="pp", bufs=2,
                                  name=f"pp_{kind}{tb}")
                proj_ps[(kind, tb)] = ps
            else:
                ps = proj_ps.pop((kind, tb))
            for kt8 in range(half * 4, half * 4 + 4):
                nc.tensor.matmul(ps, w_sb[:, kt8, :], xts[tb][:, kt8, :],
                                 start=(kt8 == 0), stop=(kt8 == D // 128 - 1))
            if half == 1:
                if kind == "q":
                    # fold bias and the 1/8 score scale into the eviction
                    nc.vector.tensor_scalar(dst[:, tq:tq + QB], ps, bq_sb,
                                            0.125, op0=OP.add, op1=OP.mult)
                else:
                    nc.vector.tensor_copy(dst[:, tq:tq + QB], ps)

        def proj(kind, b, tb):
            proj_half(kind, b, tb, 0)
            proj_half(kind, b, tb, 1)

        def vk_ones(b, half=None):
            # ones blocks: h0 cols 0:64, h1 cols 64:128 (per-kt contiguous
            # memsets; strided 3D memset semantics unverified on HW)
            rng = range(NKT) if half is None else \
                range(half * NKT // 2, (half + 1) * NKT // 2)
            for ktl in rng:
                nc.vector.memset(vk[b][0][:, ktl, 0:HD], 1.0)
                nc.vector.memset(vk[b][1][:, ktl, HD:128], 1.0)

        def vk_trans(b, tb):
            """xbar-transpose V of token block tb (local to batch b) into vk.
            One 3D-out call per head covers 4 key tiles: out[p, g, j] =
            in[j, g*128 + p]. The xbar writes a CONTIGUOUS staging tile
            (non-contiguous transpose destinations produce wrong output on
            HW -- see tile_matmul.py); a DVE copy then scatters into vk."""
            q4 = (tb % NQB) * 4
            for h, vcol in ((0, HD), (1, 0)):
                stg = nrm_pool.tile([128, 4, HD], fp16, tag="vstg", bufs=4,
                                    name=f"stg_{b}_{tb}_{h}")
                nc.sync.dma_start_transpose(
                    out=stg,
                    in_=vT[b][h * HD:(h + 1) * HD,
                              (tb % NQB) * QB:(tb % NQB + 1) * QB])
                nc.vector.tensor_copy(
                    vk[b][h][:, q4:q4 + 4, vcol:vcol + HD], stg)

        def fc_half(b, tb2, eb, evict_engine="v"):
            """FC for one 128-token tile, one 512-wide output half."""
            fp = ps_pool.tile([128, QB], f32, tag="pp", bufs=2,
                              name=f"fp_{b}_{tb2}_{eb}")
            nc.tensor.matmul(
                fp, valuesT[b][:, tb2 * 128:(tb2 + 1) * 128],
                wfc_sb[:, eb * QB:(eb + 1) * QB], start=True, stop=True)
            fo = fout_pool.tile([128, QB], f32, tag="fout",
                                name=f"fo_{b}_{tb2}_{eb}")
            if evict_engine == "s":
                nc.scalar.copy(fo, fp)
            else:
                nc.vector.tensor_copy(fo, fp)
            tt = b * S + tb2 * 128
            nc.sync.dma_start(out=OUT[tt:tt + 128, eb * QB:(eb + 1) * QB],
                              in_=fo)
            if _DEBUG:
                nc.sync.dma_start(out=DBG_FOUT[tt // 128, :, eb * QB:
                                               (eb + 1) * QB], in_=fo)

        # --- work queue: one item drained per key-tile slot (16 per step) ---
        work = deque()
        def drain(n=1):
            for _ in range(n):
                if work:
                    work.popleft()()

        def W(fn, *a):
            work.append(lambda: fn(*a))

        # NOTE on ordering: the tile framework derives dependencies from
        # ISSUE order -- issuing an instruction before the one producing its
        # input records no dependency at all (it reads uninitialized SBUF;
        # on warm reruns the stale data happens to equal the correct data,
        # masking the race). So every work item must be drained AFTER its
        # producers are issued: Q/K/V/T of batch 1 before step 4, FC of
        # (b, qb) after step (b, qb)'s normalization.

        def PH(kind, b, tb):
            W(proj_half, kind, b, tb, 0)
            W(proj_half, kind, b, tb, 1)

        # per-step static fillers, pushed at step start
        sched = [[] for _ in range(8)]

        def push_step(s):
            for item in sched[s]:
                work.append(item)

        def L(s, fn, *a):
            sched[s].append(lambda: fn(*a))

        def LPH(s, kind, b, tb):
            L(s, proj_half, kind, b, tb, 0)
            L(s, proj_half, kind, b, tb, 1)

        # step 0: rest of batch-0 proj. HARD ISSUE DEADLINES (see NOTE):
        # K block tb before score iteration 4*tb, vk transpose tb before the
        # trailing AV(4*tb) is issued, Q(qb) before step qb. All K's first,
        # then V+transpose pairs, then Q's; step 0 drains 2 items per slot.
        for tb in (1, 2, 3):
            LPH(0, "k", 0, tb)
        for tb in (1, 2, 3):
            LPH(0, "v", 0, tb)
            L(0, vk_trans, 0, tb)
        for tb in (1, 2, 3):
            LPH(0, "q", 0, tb)
        # steps 1-3: batch-1 proj + vk (all needed before step 4)
        L(1, vk_ones, 1, 0)
        L(1, vk_ones, 1, 1)
        LPH(1, "k", 1, 4); LPH(1, "v", 1, 4); L(1, vk_trans, 1, 4)
        LPH(2, "k", 1, 5); LPH(2, "v", 1, 5); L(2, vk_trans, 1, 5)
        LPH(3, "q", 1, 4)
        LPH(3, "k", 1, 6); LPH(3, "v", 1, 6); L(3, vk_trans, 1, 6)
        LPH(3, "k", 1, 7); LPH(3, "v", 1, 7); L(3, vk_trans, 1, 7)
        LPH(4, "q", 1, 5)
        LPH(5, "q", 1, 6)
        LPH(6, "q", 1, 7)
        # FC is deferred into the ACT-paced batch-1 phase (PE has slack there)
        fc_sched = {4: [(0, 0), (0, 1)], 5: [(0, 2), (0, 3)],
                    6: [(1, 0), (1, 1)], 7: [(1, 2)]}
        for s5, qbs in fc_sched.items():
            for fb, fqb in qbs:
                for tb2 in range(fqb * 4, (fqb + 1) * 4):
                    for eb in range(D // QB):
                        L(s5, fc_half, fb, tb2, eb)

        # --- prefix: minimal work to start the exp stream ---
        _pre = nc.named_scope("prefix")
        _pre.__enter__()
        proj("k", 0, 0)
        proj("q", 0, 0)
        proj("v", 0, 0)
        vk_ones(0)
        vk_trans(0, 0)
        _pre.__exit__(None, None, None)

        # --- main loop: 8 attention steps pace the kernel ---
        for s in range(B * NQB):
            b, qb = divmod(s, NQB)
            push_step(s)
            _at = nc.named_scope(f"step{s}")
            _at.__enter__()
            pav = [ps_pool.tile([128, QB], f32, tag=f"pav{h}",
                                name=f"pav{h}_{s}") for h in range(HPC)]

            def av(kt, et):
                for h in range(HPC):
                    nc.tensor.matmul(pav[h], vk[b][h][:, kt, :],
                                     et[:, h * QB:(h + 1) * QB],
                                     start=(kt == 0), stop=(kt == NKT - 1))

            pending = []
            for kt in range(NKT):
                sc = ps_pool.tile([128, 2 * QB], f32, tag="sc", bufs=2,
                                  name=f"sc_{s}_{kt}")
                for h in range(HPC):
                    hp = h * HD
                    nc.tensor.matmul(
                        sc[:, h * QB:(h + 1) * QB],
                        kT[b][hp:hp + HD, kt * KT:(kt + 1) * KT],
                        qT[b][hp:hp + HD, qb * QB:(qb + 1) * QB],
                        start=True, stop=True, tile_position=(hp, 0))
                if s >= 4 and kt == 6:
                    # Schraudolph exp on the DVE: exp(x) ~= bitcast_fp16(
                    # round(x*1477.32 + 15360)). Softmax cancels the bias;
                    # residual jitter ~1.8% on 2/16 of the keys. Offloads
                    # the ACT-bound batch-1 phase.
                    eti = et_pool.tile([128, 2 * QB], mybir.dt.int16,
                                       tag="eti", bufs=3)
                    nc.vector.tensor_scalar(eti, sc, 1477.3197, 15315.84,
                                            op0=OP.mult, op1=OP.add)
                    et = eti[:, :].bitcast(fp16)
                else:
                    et = et_pool.tile([128, 2 * QB], fp16, tag="et")
                    nc.scalar.activation(et, sc, AF.Exp)
                if _DEBUG and s == 0:
                    nc.sync.dma_start(out=DBG_ET[kt], in_=et)
                pending.append((kt, et))
                if len(pending) > AV_LAG:
                    av(*pending.pop(0))
                # step 0: light injection while the exp pipeline fills, then
                # catch up (deadlines: K03b<=iter8, T03<=iter15, all by 10)
                drain(1 if (s > 0 or kt < 2) else 3)
            for item in pending:
                av(*item)
            if _DEBUG and s == 0:
                for h in range(HPC):
                    pc = nrm_pool.tile([128, QB], f32, tag="dbgpav", bufs=2,
                                       name=f"dbgpav{h}")
                    nc.vector.tensor_copy(pc, pav[h])
                    nc.sync.dma_start(out=DBG_PAV[h], in_=pc)

            # normalization: recip of denominators, GpSimd partition-broadcast,
            # fused into the (PSUM -> fp16 SBUF) value eviction.
            # h0: den in pav0 parts 0:64 (all rows identical), values 64:128.
            r0 = nrm_pool.tile([128, QB], f32, tag="r0", name=f"r0_{s}")
            nc.vector.reciprocal_approx_fast(out=r0[0:1, :], in_=pav[0][0:1, :])
            # partition_broadcast only writes ranges based at partition 0 --
            # broadcast the full 128 and read the upper half (GpSimd is idle)
            r0b = nrm_pool.tile([128, QB], f32, tag="r0b", name=f"r0b_{s}")
            nc.gpsimd.partition_broadcast(r0b, r0[0:1, :])
            nc.vector.tensor_mul(valuesT[b][HD:128, qb * QB:(qb + 1) * QB],
                                 pav[0][HD:128, :], r0b[HD:128, :])
            # h1: values in parts 0:64, den in 64:128 -> row-move via DMA.
            r1 = nrm_pool.tile([128, QB], f32, tag="r1", name=f"r1_{s}")
            nc.vector.tensor_copy(r1[HD:HD + 1, :], pav[1][HD:HD + 1, :])
            r1b = nrm_pool.tile([128, QB], f32, tag="r1b", name=f"r1b_{s}")
            nc.sync.dma_start(out=r1b[0:1, :], in_=r1[HD:HD + 1, :])
            rec1 = nrm_pool.tile([128, QB], f32, tag="rec1", name=f"rec1_{s}")
            nc.vector.reciprocal_approx_fast(out=rec1[0:1, :], in_=r1b[0:1, :])
            rec1b = nrm_pool.tile([128, QB], f32, tag="rec1b", name=f"rec1b_{s}")
            nc.gpsimd.partition_broadcast(rec1b[0:HD, :], rec1[0:1, :])
            nc.vector.tensor_mul(valuesT[b][0:HD, qb * QB:(qb + 1) * QB],
                                 pav[1][0:HD, :], rec1b[0:HD, :])
            if _DEBUG and s == 0:
                for h in range(HPC):
                    nc.sync.dma_start(out=DBG_VK[h], in_=vk[0][h])
                nc.sync.dma_start(out=DBG_KQV[0], in_=kT[0])
                nc.sync.dma_start(out=DBG_KQV[1], in_=qT[0])
                nc.sync.dma_start(out=DBG_KQV[2], in_=vT[0])
            if _DEBUG:
                nc.sync.dma_start(out=DBG_DEN[s, 0, :], in_=r1b[0:1, :])
                nc.sync.dma_start(out=DBG_DEN[s, 1, :], in_=r0b[64:65, :])
                nc.sync.dma_start(out=DBG_DEN[s, 2, :], in_=rec1b[0:1, :])
                nc.sync.dma_start(
                    out=DBG_V[b, :, qb * QB:(qb + 1) * QB],
                    in_=valuesT[b][:, qb * QB:(qb + 1) * QB])
            _at.__exit__(None, None, None)

        # tail: leftover queue items, then the last FC block
        drain(len(work))
        for tb2 in range(12, 16):
            for eb in range(D // QB):
                fc_half(1, tb2, eb)

    nc.compile()
    return nc


def _get_nc():
    global _NC_CACHE
    if _NC_CACHE is None:
        _NC_CACHE = _build()
    return _NC_CACHE


def _prep_in_maps(x, w_qkv, b_qkv, w_fc):
    # x -> [tb, p, kt, t] so each block's partition line is 8KB-contiguous
    xT = x.reshape(T, D).T.astype(np.float16)          # [D, T]
    xb = np.ascontiguousarray(
        xT.reshape(D // 128, 128, NTB, QB).transpose(2, 1, 0, 3))
    in_maps = []
    for c in range(NC):
        heads = [HPC * c + i for i in range(HPC)]
        rq = np.concatenate([np.arange(h * 3 * HD, h * 3 * HD + HD) for h in heads])
        rk = np.concatenate([np.arange(h * 3 * HD + HD, h * 3 * HD + 2 * HD) for h in heads])
        rv = np.concatenate([np.arange(h * 3 * HD + 2 * HD, h * 3 * HD + 3 * HD) for h in heads])

        def wprep(rows):
            # [D, F] -> [kt, p, F] -> SBUF tile [p, kt, F]
            wt = w_qkv[rows].T.astype(np.float16)      # [D, F]
            return np.ascontiguousarray(
                wt.reshape(D // 128, 128, F).transpose(1, 0, 2))

        # valuesT rows: 0:64 = head1 features, 64:128 = head0 features
        d1 = np.arange(heads[1] * HD, heads[1] * HD + HD)
        d0 = np.arange(heads[0] * HD, heads[0] * HD + HD)
        dperm = np.concatenate([d1, d0])
        m = {
            "xb": xb,
            "wq": wprep(rq),
            "wk": wprep(rk),
            "wv": wprep(rv),
            "bq": np.ascontiguousarray(b_qkv[rq][:, None].astype(np.float32)),
            "wfc": np.ascontiguousarray(w_fc[:, dperm].T.astype(np.float16)),
        }
        in_maps.append(m)
    return in_maps


def run_kernel(inputs, trace=False, trace_cores=None):
    x = np.asarray(inputs["x"], np.float32)
    w_qkv = np.asarray(inputs["w_qkv"], np.float32)
    b_qkv = np.asarray(inputs["b_qkv"], np.float32)
    w_fc = np.asarray(inputs["w_fc"], np.float32)
    b_fc = np.asarray(inputs["b_fc"], np.float32)

    nc = _get_nc()
    in_maps = _prep_in_maps(x, w_qkv, b_qkv, w_fc)
    res = run_bass_kernel_spmd(
        nc, in_maps, core_ids=list(range(NC)), trace=trace,
        trace_cores=trace_cores,
    )
    out = res.results[0]["out"].astype(np.float32)
    for r in res.results[1:]:
        out = out + r["out"]
    # V bias passes through attention unchanged; fold bv @ w_fc^T into the
    # host-side bias add (exact math, not an approximation).
    rv_all = np.concatenate(
        [np.arange(h * 3 * HD + 2 * HD, h * 3 * HD + 3 * HD) for h in range(H)])
    bv_full = b_qkv[rv_all]
    out = out + b_fc[None, :] + (w_fc @ bv_full)[None, :]
    return out.reshape(B, S, D), res


def kernel(**inputs):
    out, _ = run_kernel(inputs, trace=False)
    return out

